# revision 67
# baseline (speedup 1.0000x reference)
"""Trainium2 Bass kernel for the attention-based encoder (bf16 v2).

Computation (per batch b):
    a      = P @ y[b]                                  # [D]
    logits = x[b] @ a                                  # [M]
    p_un   = exp(logits - 16); Z = sum(p_un)
    W[m]   = p_un[m-1] + p_un[m] + p_un[m+1] + p_un[m+2]  (zero-padded), W[M-1] = 0
    enc[b] = (W @ x[b]) / (Q * Z)                      # [D]

Identical math to the reference (cumsum sliding window + bilinear softmax),
with the smoothing window folded onto the softmax weights.

Design (vs the fp32 baseline, ~385us -> ~225us):
  - all big operands (x, P^T, y) are cast to bf16 on the host: halves HBM
    traffic (the bottleneck regime) and runs PE matmuls at 1 cyc/row
    instead of fp32's 4.
  - m-layout is partition-major (m = p*16 + j): x loads become fully
    contiguous 16KB-per-partition chunk DMAs, and the 4-tap window
    W = S4 @ p becomes 6 gpsimd shifted adds in the free dim plus two
    partition-shifted SBUF->SBUF halo DMAs instead of 46 PE banded
    matmuls per batch.
  - logits (x . a) work is split three ways per batch: 10 js multiply on
    DVE + reduce on ACT (Copy with accumulate), 6 js multiply + reduce
    on DVE, 4 of the muls go to gpsimd — balancing all engines under the
    DMA roofline.
  - a-replication via row-selector matmuls (no partition-0 bounce DMA);
    Z via a ones-column matmul placed after the PE tail; the whole
    epilogue (1/2Z scale + enc DMA) is deferred two batches at demoted
    scheduler priority so the in-order ACT/DVE queues never stall on the
    PE tail; x loads for all batches are issued up front; deep tile pools
    decouple the producer/consumer engine pairs.

Sharding: data-parallel over batch, 4 batches per core on 8 cores; P is
replicated (pre-transposed on host so the contraction dim lands on SBUF
partitions).  A sharded-P variant with an 8-core ReduceScatter was built
and validated in CoreSim, but this container's walrus rejects the
required ops (visitInstISA) at NEFF codegen, so it is not used.
"""

import numpy as np
import ml_dtypes

import concourse.bass as bass
import concourse.mybir as mybir
from concourse.tile import TileContext
from concourse.bass_utils import run_bass_kernel_spmd

# ---------------------------------------------------------------------------
# This container's walrus supports only ONE sync wait per instruction ("Too
# many sync wait commands" at codegen otherwise), while Tile freely attaches
# several.  Post-pass: hoist excess waits onto injected same-engine NoOps
# placed immediately before the over-subscribed instruction.
# ---------------------------------------------------------------------------

_MAX_WAITS = 1


def split_sync_waits(nc: bass.Bass) -> None:
    uid = 0
    for fn in nc.m.functions:
        for blk in fn.blocks:
            new_insts = []
            for inst in blk.instructions:
                si = inst.sync_info
                waits = list(si.on_wait) if si and si.on_wait else []
                if len(waits) > _MAX_WAITS:
                    for w in waits[:-_MAX_WAITS]:
                        uid += 1
                        ev = mybir.InstEventSemaphore(
                            name=f"{inst.name}_hw{uid}",
                            opcode="EventSemaphore",
                            ins=[],
                            outs=[],
                            sync_info=mybir.SyncInfo(on_wait=[w], on_update=[]),
                        )
                        ev.engine = inst.engine
                        new_insts.append(ev)
                    si.on_wait = waits[-_MAX_WAITS:]
                new_insts.append(inst)
            blk.instructions[:] = new_insts

# ---------------------------------------------------------------------------

B, M, D, CD = 32, 2048, 1024, 5120
Q = 2
NCORES = 8
BPC = B // NCORES          # batches per core
NJ = M // 128              # j-positions per partition (m = p*NJ + j)
KT = CD // 128             # k-tiles of the P contraction
F32 = mybir.dt.float32
BF16 = mybir.dt.bfloat16
ALU = mybir.AluOpType
AFT = mybir.ActivationFunctionType
AXL = mybir.AxisListType

# logits work split: per batch, js in ACT_JS multiply on DVE (or gpsimd for
# POOL_JS) and reduce on ACT (Copy+accum); the remaining js run fused
# multiply+reduce on DVE (tensor_tensor_reduce) when USE_FUSED_TTR, else
# mul + tensor_reduce.  ACT js come first so per-batch stragglers don't sit
# behind ACT's 1.2us-per-tile cadence.
XCHUNK = 4                             # js per x DMA chunk
USE_FUSED_TTR = False                  # this walrus rejects InstISA TTR on HW
if USE_FUSED_TTR:
    ACT_JS = frozenset(range(8))       # 8 ACT reduces / 8 DVE fused js
    POOL_JS = frozenset((1, 3, 5, 7))  # 4 gpsimd muls per batch
else:
    ACT_JS = frozenset(range(10))      # 10 ACT reduces / 6 DVE mul+reduce js
    POOL_JS = frozenset((1, 3, 5, 7, 9))  # 5 gpsimd muls per batch

# Phase A sharded over cores: each core contracts its 640-row slice of P^T
# for ALL 32 batches, then an 8-core DRAM AllReduce combines the partial
# a-vectors (128KB) — replaces 10MB/core of replicated P traffic.
USE_COLLECTIVE = False
COLLECTIVE_BYPASS = False              # True: plain copy (CoreSim debugging)
KTL = KT // NCORES                     # local k-tiles (5)
CDL = CD // NCORES                     # local contraction rows (640)


def build_nc(reps: int = 1, n_batches: int = BPC) -> bass.Bass:
    nc = bass.Bass()
    xs = nc.declare_dram_parameter("xs", [BPC, M, D], BF16, isOutput=False)
    pt = nc.declare_dram_parameter("pt", [CD, D], BF16, isOutput=False)
    ys = nc.declare_dram_parameter("ys", [128, KT, BPC], BF16, isOutput=False)
    enc = nc.declare_dram_parameter("enc", [BPC, D], F32, isOutput=True)

    with TileContext(nc) as tc:
        with (
            tc.tile_pool(name="const", bufs=1) as const_pool,
            tc.tile_pool(name="ptp", bufs=6) as pt_pool,
            tc.tile_pool(name="xp", bufs=4) as x_pool,
            tc.tile_pool(name="arep", bufs=1) as arep_pool,
            tc.tile_pool(name="small", bufs=2) as small_pool,
            tc.tile_pool(name="tiny", bufs=12) as tiny_pool,
            tc.tile_pool(name="scr", bufs=10) as scr_pool,
            tc.tile_pool(name="psh", bufs=1, space="PSUM") as psum_h_pool,
            tc.tile_pool(name="pse", bufs=3, space="PSUM") as psum_e_pool,
        ):
            ones_col = const_pool.tile([128, 1], F32)
            nc.vector.memset(ones_col[:], 1.0)
            # row-selector weights: sel[b][p, i] = 1 iff p == b, so that
            # matmul(lhsT=sel[b], rhs=aT) broadcasts aT row b to all 128
            # partitions without any partition-0 bounce DMA
            sel = []
            for b in range(BPC):
                s = const_pool.tile([BPC, 128], BF16, tag=f"sel{b}")
                nc.gpsimd.memset(s[:], 0.0)
                nc.gpsimd.affine_select(
                    out=s[:], in_=s[:], compare_op=ALU.not_equal, fill=1.0,
                    base=-b, pattern=[[0, 128]], channel_multiplier=1,
                )
                sel.append(s)
            nshift = const_pool.tile([128, 1], F32)
            nc.vector.memset(nshift[:], -16.0)
            ys_sb = const_pool.tile([128, KT, BPC], BF16)
            nc.sync.dma_start(out=ys_sb[:], in_=ys[:])

            a_rep = [
                arep_pool.tile([128, D], BF16, tag=f"a_rep{b}", name=f"a_rep{b}")
                for b in range(BPC)
            ]

            def body(_=None):
                # ---- Phase A: aT[b, d] = sum_k y[b, k] * PT[k, d] ----
                pa0 = psum_e_pool.tile([BPC, 512], F32, tag="pe0")
                pa1 = psum_e_pool.tile([BPC, 512], F32, tag="pe1")
                for c in range(KT // 2):
                    ptt = pt_pool.tile([128, 2, D], BF16, tag="ptt")
                    nc.sync.dma_start(
                        out=ptt[:],
                        in_=pt[c * 256:(c + 1) * 256, :].rearrange(
                            "(u p) d -> p u d", p=128),
                    )
                    for u in range(2):
                        t = c * 2 + u
                        for dh, pa in enumerate((pa0, pa1)):
                            nc.tensor.matmul(
                                pa[:],
                                lhsT=ys_sb[:, t, :],
                                rhs=ptt[:, u, dh * 512:(dh + 1) * 512],
                                start=(t == 0),
                                stop=(t == KT - 1),
                            )
                aT_sb = small_pool.tile([BPC, D], BF16, tag="aT")
                nc.scalar.copy(out=aT_sb[:, 0:512], in_=pa0[:])
                nc.scalar.copy(out=aT_sb[:, 512:1024], in_=pa1[:])

                # replicate a[b] across all 128 partitions (selector matmul)
                for b in range(BPC):
                    for dh in range(2):
                        pr = psum_e_pool.tile([128, 512], F32, tag="pe0")
                        nc.tensor.matmul(
                            pr[:],
                            lhsT=sel[b][:],
                            rhs=aT_sb[:, dh * 512:(dh + 1) * 512],
                            start=True,
                            stop=True,
                        )
                        nc.scalar.copy(
                            out=a_rep[b][:, dh * 512:(dh + 1) * 512], in_=pr[:]
                        )

                # ---- Phase B: x loads for ALL batches issued up front so no
                # output/bounce DMA ever blocks a load on the SP queue ----
                xbs = []
                for b in range(n_batches):
                    xb = x_pool.tile([128, NJ, D], BF16, tag="xb")
                    xbs.append(xb)
                    xsrc = xs[b, :, :].rearrange("(p j) d -> p j d", p=128)
                    for h in range(NJ // XCHUNK):
                        nc.sync.dma_start(
                            out=xb[:, XCHUNK * h:XCHUNK * (h + 1), :],
                            in_=xsrc[:, XCHUNK * h:XCHUNK * (h + 1), :],
                        )

                # ---- per-batch attention ----
                # The epilogue (PSUM->SBUF scale + enc DMA) of batch b is
                # issued during batch b+1's section so the in-order ACT queue
                # never stalls waiting for b's PE tail matmuls.
                pending = []
                z_ps = psum_h_pool.tile([1, 2 * BPC], F32, tag="z_ps")

                def flush_epilogue(keep: int = 0):
                    # demoted priority: the scheduler must not hoist these
                    # ahead of later batches' compute (they wait on the slow
                    # PE tail and would head-of-line-block the ACT queue)
                    with tc.high_priority(offset=-1000000):
                        while len(pending) > keep:
                            b_, pe0_, pe1_ = pending.pop(0)
                            # z2 = Q * (Z_half0 + Z_half1) via scaled accum
                            zd = tiny_pool.tile([1, 2], F32, tag="zd")
                            z2 = tiny_pool.tile([1, 1], F32, tag="z2")
                            nc.scalar.activation(
                                out=zd[:],
                                in_=z_ps[0:1, 2 * b_:2 * b_ + 2],
                                func=AFT.Copy,
                                scale=float(Q),
                                accum_out=z2[:],
                            )
                            rz = tiny_pool.tile([1, 1], F32, tag="rz")
                            nc.vector.reciprocal(rz[:], z2[:])
                            enc_sb = small_pool.tile([1, D], F32, tag="enc_sb")
                            nc.scalar.activation(
                                out=enc_sb[:, 0:512], in_=pe0_[:], func=AFT.Copy,
                                scale=rz[:],
                            )
                            nc.scalar.activation(
                                out=enc_sb[:, 512:1024], in_=pe1_[:],
                                func=AFT.Copy, scale=rz[:],
                            )
                            nc.sync.dma_start(out=enc[b_, :], in_=enc_sb[0:1, :])

                for b in range(n_batches):
                    xb = xbs[b]

                    # logits[p, j] = x[m, :] . a   (m = p*NJ + j)
                    logits_sb = tiny_pool.tile([128, NJ], F32, tag="logits")
                    for j in range(NJ):
                        scratch = scr_pool.tile([128, D], BF16, tag="scratch")
                        if j in ACT_JS:
                            eng = nc.gpsimd if j in POOL_JS else nc.vector
                            eng.tensor_mul(scratch[:], xb[:, j, :], a_rep[b][:])
                            nc.scalar.activation(
                                out=scratch[:],
                                in_=scratch[:],
                                func=AFT.Copy,
                                accum_out=logits_sb[:, j:j + 1],
                            )
                        elif USE_FUSED_TTR:
                            nc.vector.tensor_tensor_reduce(
                                out=scratch[:],
                                in0=xb[:, j, :],
                                in1=a_rep[b][:],
                                scale=1.0,
                                scalar=0.0,
                                op0=ALU.mult,
                                op1=ALU.add,
                                accum_out=logits_sb[:, j:j + 1],
                            )
                        else:
                            nc.vector.tensor_mul(
                                scratch[:], xb[:, j, :], a_rep[b][:]
                            )
                            # fold 1024 -> 256 in bf16 2x mode, then one
                            # short fp32-rate reduce: ~730ns vs 1127ns
                            nc.vector.tensor_add(
                                scratch[:, 0:512], scratch[:, 0:512],
                                scratch[:, 512:1024],
                            )
                            nc.vector.tensor_add(
                                scratch[:, 0:256], scratch[:, 0:256],
                                scratch[:, 256:512],
                            )
                            nc.vector.tensor_reduce(
                                out=logits_sb[:, j:j + 1],
                                in_=scratch[:, 0:256],
                                axis=AXL.X,
                                op=ALU.add,
                            )

                    # softmax numerator with a FIXED shift (cancels in the
                    # final scale): p_un = exp(logits - 16).  exp is SPLIT at
                    # the ACT/DVE logits boundary: exp0 (js 0..9) fires as
                    # soon as ACT's own reduces land, unlocking W cols 1..7
                    # and their PE tail matmuls while DVE still computes
                    # js 10..15.  Z accumulates per half into zcol cols.
                    NA = 10  # = len(ACT_JS)
                    pp = tiny_pool.tile([128, NJ], F32, tag="pp")
                    zcol = tiny_pool.tile([128, 2], F32, tag=f"zcol{b}")
                    nc.scalar.activation(
                        out=pp[:, 0:NA],
                        in_=logits_sb[:, 0:NA],
                        func=AFT.Exp,
                        bias=nshift[:],
                        scale=1.0,
                        accum_out=zcol[:, 0:1],
                    )
                    # halo[p, 0:2] = pp[p+1, 0:2] (needs exp0 only)
                    halo = tiny_pool.tile([128, 3], F32, tag="halo")
                    nc.gpsimd.memset(halo[:], 0.0)
                    nc.sync.dma_start(out=halo[0:127, 0:2], in_=pp[1:128, 0:2])

                    # W half A: cols 1..7 (pp 0..9 only)
                    w = tiny_pool.tile([128, NJ], F32, tag="w")
                    nc.gpsimd.tensor_add(w[:, 1:8], pp[:, 1:8], pp[:, 2:9])
                    nc.gpsimd.tensor_add(w[:, 1:8], w[:, 1:8], pp[:, 3:10])
                    nc.gpsimd.tensor_add(w[:, 1:8], w[:, 1:8], pp[:, 0:7])
                    w_pm = tiny_pool.tile([128, NJ], BF16, tag="w_pm")
                    nc.gpsimd.tensor_copy(out=w_pm[:, 1:8], in_=w[:, 1:8])

                    pe0 = psum_e_pool.tile([1, 512], F32, tag="pe0")
                    pe1 = psum_e_pool.tile([1, 512], F32, tag="pe1")

                    def tail_mm(j, start, stop):
                        pend = 127 if j == NJ - 1 else 128
                        for dh, pe in enumerate((pe0, pe1)):
                            nc.tensor.matmul(
                                pe[:],
                                lhsT=w_pm[0:pend, j:j + 1],
                                rhs=xb[0:pend, j, dh * 512:(dh + 1) * 512],
                                start=start,
                                stop=stop,
                            )

                    for j in range(1, 8):
                        tail_mm(j, start=(j == 1), stop=False)

                    # exp1 over the DVE js, then halo prv + W half B
                    nc.scalar.activation(
                        out=pp[:, NA:NJ],
                        in_=logits_sb[:, NA:NJ],
                        func=AFT.Exp,
                        bias=nshift[:],
                        scale=1.0,
                        accum_out=zcol[:, 1:2],
                    )
                    # halo[p, 2] = pp[p-1, 15] (needs exp1)
                    nc.sync.dma_start(
                        out=halo[1:128, 2:3], in_=pp[0:127, NJ - 1:NJ]
                    )
                    # W half B: cols 8..15 and col 0
                    nc.gpsimd.tensor_add(w[:, 8:16], pp[:, 8:16], pp[:, 7:15])
                    nc.gpsimd.tensor_add(w[:, 8:15], w[:, 8:15], pp[:, 9:16])
                    nc.gpsimd.tensor_add(w[:, 15:16], w[:, 15:16], halo[:, 0:1])
                    nc.gpsimd.tensor_add(w[:, 8:14], w[:, 8:14], pp[:, 10:16])
                    nc.gpsimd.tensor_add(w[:, 14:16], w[:, 14:16], halo[:, 0:2])
                    nc.gpsimd.tensor_add(w[:, 0:1], pp[:, 0:1], pp[:, 1:2])
                    nc.gpsimd.tensor_add(w[:, 0:1], w[:, 0:1], pp[:, 2:3])
                    nc.gpsimd.tensor_add(w[:, 0:1], w[:, 0:1], halo[:, 2:3])
                    nc.gpsimd.tensor_copy(out=w_pm[:, 8:16], in_=w[:, 8:16])
                    nc.gpsimd.tensor_copy(out=w_pm[:, 0:1], in_=w[:, 0:1])

                    tail_mm(0, start=False, stop=False)
                    for j in range(8, NJ):
                        tail_mm(j, start=False, stop=(j == NJ - 1))

                    # Z halves -> shared PSUM tile (ones-column matmul after
                    # the tail in PE queue order); epilogue sums the halves.
                    nc.tensor.matmul(z_ps[0:1, 2 * b:2 * b + 2],
                                     lhsT=ones_col[:], rhs=zcol[:, 0:2],
                                     start=True, stop=True)

                    # enc[b] = enc_un / (Q * Z): deferred two batches via
                    # flush_epilogue so it never blocks the hot ACT/DVE queues
                    pending.append((b, pe0, pe1))
                    flush_epilogue(keep=1)

                flush_epilogue()

            if reps == 1:
                body()
            else:
                with tc.For_i(0, reps, 1):
                    body()

    return nc


def _shard_inputs(embeds_x, embeds_y, P):
    """Build the 8 per-core input maps (host-side resharding + bf16 cast)."""
    x = np.asarray(embeds_x, dtype=np.float32).astype(ml_dtypes.bfloat16)
    y = np.asarray(embeds_y, dtype=np.float32)[:, :, 0].astype(ml_dtypes.bfloat16)
    pt = np.ascontiguousarray(
        np.asarray(P, dtype=np.float32).T.astype(ml_dtypes.bfloat16)
    )  # [CD, D]
    in_maps = []
    for c in range(NCORES):
        sl = slice(c * BPC, (c + 1) * BPC)
        ys_c = np.ascontiguousarray(
            y[sl].reshape(BPC, KT, 128).transpose(2, 1, 0)
        )  # [128, KT, BPC]
        in_maps.append({
            "xs": np.ascontiguousarray(x[sl]),
            "pt": pt,
            "ys": ys_c,
        })
    return in_maps


def kernel(embeds_x, embeds_y, P, M):
    assert int(M) == 2048
    nc = build_nc(reps=1)
    split_sync_waits(nc)  # HW-compile only; CoreSim rejects injected NoOps
    in_maps = _shard_inputs(embeds_x, embeds_y, P)
    res = run_bass_kernel_spmd(nc, in_maps, list(range(NCORES)))
    out = np.concatenate([res.results[c]["enc"] for c in range(NCORES)], axis=0)
    return out.astype(np.float32)
